# revision 26
# baseline (speedup 1.0000x reference)
"""Trainium2 Bass kernel for nn_MegaLocMPS (retrieval_knn).

Reference computation:
    h      = relu(local_features @ W1_local + b1_local)            [B,N,mlp]
    local  = softmax(h @ W2_local + b2_local, axis=N).sum(axis=N)  [B,K*C]
    g      = relu(global_token @ W1_global + b1) @ W2_global + b2  [B,td]
    feats  = l2_normalize(concat([local, g]))                      [B,16640]
    sim    = feats @ feats.T ; mask = (sim > 0.7) | topk(sim, k) | diag

Key algebraic identity: softmax(x, axis=1).sum(axis=1) == 1 exactly, for any
input — so `local` is identically ones(B, 16384) and the whole local branch
(W1_local/W2_local, the 137-GMAC matmul) contributes only the constant 1.0.
Therefore:
    norm_b = sqrt(16384 + ||g_b||^2)
    feats  = [1/norm_b broadcast 16384 | g_b/norm_b]
    sim_ij = (16384 + g_i.g_j) * inv_i * inv_j
           = a_i . a_j  with a_i = [g_i*inv_i, 128*inv_i]   (257-dim)
The device kernel computes the global MLP, norms, sim, threshold, and an exact
top-k mask (top-8 `max` + `match_replace` + threshold at the k-th value).

Sharding: the tiny compute is replicated on all 8 cores; the large feats
output write ([64,16640] = 4.3 MB) is row-sharded 8 ways via per-core one-hot
selection matrices (SPMD-friendly: same program, different data).
"""

import os
import sys

for _p in ("/opt/trn_rl_repo", "/root/.axon_site/_ro/trn_rl_repo"):
    if os.path.isdir(_p) and _p not in sys.path:
        sys.path.append(_p)

import numpy as np

import concourse.bass as bass
import concourse.mybir as mybir
import concourse.tile as tile
from concourse import bacc
from concourse.bass_utils import run_bass_kernel_spmd
from concourse.masks import make_identity

F32 = mybir.dt.float32

B = 64          # batch (images)
D = 768         # input feature dim
MLP = 512       # hidden dim
TD = 256        # global token output dim
KC = 16384      # NUM_CLUSTERS * CLUSTER_DIM (all-ones part of feats)
KD = D // 128   # 6 contraction chunks for layer 1
KM = MLP // 128 # 4 chunks for layer 2
N_CORES = 8
RPC = B // N_CORES  # 8 output rows per core
THRESHOLD = 0.7

_cache = {}


def _build(k: int):
    """Build + compile the SPMD program (same on all cores)."""
    nc = bacc.Bacc("TRN2", debug=False, num_devices=N_CORES)

    # Inputs (host pre-tiled so every DMA is partition-contiguous).
    # small = [b2r (256) | rsel (128) | sel (8)] packed into one [64, 392].
    gtT_d = nc.dram_tensor("gtT", [128, KD, B], F32, kind="ExternalInput").ap()
    w1_d = nc.dram_tensor("w1", [128, KD, MLP], F32, kind="ExternalInput").ap()
    b1_d = nc.dram_tensor("b1", [128, KM], F32, kind="ExternalInput").ap()
    w2_d = nc.dram_tensor("w2", [128, KM, TD], F32, kind="ExternalInput").ap()
    small_d = nc.dram_tensor("small", [B, TD + 128 + RPC], F32,
                             kind="ExternalInput").ap()

    feats_d = nc.dram_tensor("feats", [RPC, KC + TD], F32, kind="ExternalOutput").ap()
    mask_d = nc.dram_tensor("mask", [B, B], F32, kind="ExternalOutput").ap()

    with tile.TileContext(nc) as tc:
        with (
            tc.tile_pool(name="const", bufs=1) as cpool,
            tc.tile_pool(name="work", bufs=1) as wpool,
            tc.tile_pool(name="psum", bufs=1, space="PSUM") as ppool,
        ):
            # ---- constants (gpsimd, off the DMA/compute path) -------------
            ident = cpool.tile([B, B], F32)
            make_identity(nc, ident)
            kc_const = cpool.tile([B, 1], F32)
            nc.gpsimd.memset(kc_const, float(KC))
            dumm = cpool.tile([1, 1], F32)
            nc.gpsimd.memset(dumm, 4.0)
            ones1 = cpool.tile([1, B], F32)
            nc.gpsimd.memset(ones1, 1.0)

            # ---- input DMAs, issue split across the two HWDGE engines -----
            # sync queue: gtT, w1 chunks {0,1}/{4,5}, b1
            # scalar queue: w1 chunks {2,3}, w2, packed small tile
            gtT_sb = cpool.tile([128, KD, B], F32)
            nc.sync.dma_start(gtT_sb, gtT_d)
            w1_sb = cpool.tile([128, KD, MLP], F32)
            nc.sync.dma_start(w1_sb[:, 0:1, :], w1_d[:, 0:1, :])
            nc.sync.dma_start(w1_sb[:, 1:3, :], w1_d[:, 1:3, :])
            nc.sync.dma_start(w1_sb[:, 3:6, :], w1_d[:, 3:6, :])
            b1_sb = cpool.tile([128, KM], F32)
            nc.sync.dma_start(b1_sb, b1_d)
            w2_sb = cpool.tile([128, KM, TD], F32)
            nc.sync.dma_start(w2_sb, w2_d)
            small_sb = cpool.tile([B, TD + 128 + RPC], F32)
            nc.sync.dma_start(small_sb, small_d)
            b2r_sb = small_sb[:, 0:TD]
            rsel_sb = small_sb[:, TD:TD + 128]
            sel_sb = small_sb[:, TD + 128:]

            # pre-warm both ACT function tables while the PE works
            # (Sqrt lives in a second table; loading it lazily would stall
            # the norm chain ~1.3us). Scheduled after scalar's DMA issues.
            dumo = cpool.tile([1, 1], F32)
            with tc.high_priority():
                nc.scalar.activation(dumo, dumm,
                                     mybir.ActivationFunctionType.Relu)
                nc.scalar.activation(dumo, dumm,
                                     mybir.ActivationFunctionType.Sqrt)

            # ---- layer 1: hT[m] = relu(W1[:,m].T @ gt.T + b1[m]) ----------
            # m processed in groups of two, k-outer within a group, so the
            # first matmul only needs w1 chunk 0 (overlaps the w1 DMA) and
            # group A's relu overlaps group B's matmuls.
            hT_sb = wpool.tile([128, KM, B], F32)
            hps = [
                ppool.tile([128, B], F32, tag=f"hps{m}", bufs=1,
                           name=f"hps{m}")
                for m in range(KM)
            ]
            for kk in range(KD):
                for m in range(KM):
                    nc.tensor.matmul(
                        hps[m],
                        lhsT=w1_sb[:, kk, 128 * m:128 * (m + 1)],
                        rhs=gtT_sb[:, kk, :],
                        start=(kk == 0),
                        stop=(kk == KD - 1),
                    )
            for m in range(KM):
                nc.scalar.activation(
                    hT_sb[:, m, :], hps[m],
                    mybir.ActivationFunctionType.Relu,
                    bias=b1_sb[:, m:m + 1],
                )

            # ---- layer 2: g = hT.T @ W2 + b2  [B, TD] ---------------------
            gps = ppool.tile([B, TD], F32, tag="acc", bufs=1)
            for m in range(KM):
                nc.tensor.matmul(
                    gps,
                    lhsT=hT_sb[:, m, :],
                    rhs=w2_sb[:, m, :],
                    start=(m == 0),
                    stop=False,
                )
            # bias folded in as a rank-1 accumulate: ones.T @ b2_row
            nc.tensor.matmul(gps, lhsT=ones1, rhs=b2r_sb[0:1, :],
                             start=False, stop=True)

            # ---- norms: norm = sqrt(16384 + sum(g^2)) ---------------------
            gsq = wpool.tile([B, TD], F32)
            ss = wpool.tile([B, 1], F32)
            nc.scalar.activation(
                gsq, gps, mybir.ActivationFunctionType.Square, accum_out=ss
            )
            norm = wpool.tile([B, 1], F32)
            nc.scalar.activation(
                norm, ss, mybir.ActivationFunctionType.Sqrt, bias=kc_const
            )
            inv = wpool.tile([B, 1], F32)
            nc.vector.reciprocal(inv, norm)
            inv128 = wpool.tile([B, 1], F32)
            nc.scalar.mul(inv128, inv, float(np.sqrt(KC)))

            gs = wpool.tile([B, TD], F32)  # g / norm  (the feats tail)
            nc.vector.tensor_scalar(
                gs, gps, inv, None, op0=mybir.AluOpType.mult
            )

            # ---- similarity: sim = A A.T with a_i=[gs_i, 128*inv_i] -------
            gsT_sb = wpool.tile([128, 2, B], F32)
            for j in range(2):
                tp = ppool.tile([128, B], F32, tag="tp", bufs=2)
                nc.tensor.transpose(tp, gs[:, 128 * j:128 * (j + 1)], ident)
                nc.scalar.copy(gsT_sb[:, j, :], tp)
            ivp = ppool.tile([1, B], F32, tag="small", bufs=1, name="ivp")
            nc.tensor.transpose(ivp, inv128, ident)
            invT_sb = wpool.tile([1, B], F32)
            nc.scalar.copy(invT_sb, ivp)

            sps = ppool.tile([B, B], F32, tag="acc", bufs=1, name="sps")
            nc.tensor.matmul(sps, lhsT=gsT_sb[:, 0, :], rhs=gsT_sb[:, 0, :],
                             start=True, stop=False)
            nc.tensor.matmul(sps, lhsT=gsT_sb[:, 1, :], rhs=gsT_sb[:, 1, :],
                             start=False, stop=False)
            nc.tensor.matmul(sps, lhsT=invT_sb, rhs=invT_sb,
                             start=False, stop=True)

            # ---- feats output (row shard) ---------------------------------
            # constant region: feats[b, :16384] = 1/norm_b
            irp = ppool.tile([128, 1], F32, tag="small", bufs=1)
            nc.tensor.matmul(irp, lhsT=rsel_sb, rhs=inv, start=True, stop=True)
            ir_sb = wpool.tile([128, 1], F32)
            nc.vector.tensor_copy(ir_sb, irp)
            fconst = wpool.tile([128, KC // 16], F32)
            nc.vector.tensor_copy(fconst, ir_sb.to_broadcast([128, KC // 16]))
            nc.sync.dma_start(feats_d[0:RPC // 2, 0:KC], fconst[0:64, :])
            nc.scalar.dma_start(feats_d[RPC // 2:, 0:KC], fconst[64:128, :])
            # g region: feats[b, 16384:] = g_b/norm_b for this core's rows
            gshp = ppool.tile([RPC, TD], F32, tag="small", bufs=1, name="gshp")
            nc.tensor.matmul(gshp, lhsT=sel_sb, rhs=gs, start=True, stop=True)
            gsh_sb = wpool.tile([RPC, TD], F32)
            nc.scalar.copy(gsh_sb, gshp)
            nc.sync.dma_start(feats_d[:, KC:], gsh_sb)

            # ---- mask: threshold | topk | topk.T --------------------------
            s_sb = wpool.tile([B, B], F32)
            nc.vector.tensor_copy(s_sb, sps)
            kk = min(k, B)
            if kk > 0:
                cur = s_sb
                rounds = (kk - 1) // 8
                for r in range(rounds):
                    mx = wpool.tile([B, 8], F32, tag="mx", bufs=2)
                    nc.vector.max(out=mx, in_=cur)
                    nxt = wpool.tile([B, B], F32, tag="scur", bufs=2)
                    nc.vector.match_replace(
                        out=nxt, in_to_replace=mx, in_values=cur,
                        imm_value=-10.0,
                    )
                    cur = nxt
                mxf = wpool.tile([B, 8], F32)
                nc.vector.max(out=mxf, in_=cur)
                pos = (kk - 1) % 8
                tmask = wpool.tile([B, B], F32)
                nc.vector.tensor_scalar(
                    tmask, s_sb, mxf[:, pos:pos + 1], None,
                    op0=mybir.AluOpType.is_ge,
                )
                ttp = ppool.tile([B, B], F32, tag="tp", bufs=2)
                nc.tensor.transpose(ttp, tmask, ident)
                # m1 = max(sim > 0.7, topk)   (one fused DVE op)
                m1 = wpool.tile([B, B], F32)
                nc.vector.scalar_tensor_tensor(
                    out=m1, in0=s_sb, scalar=THRESHOLD, in1=tmask,
                    op0=mybir.AluOpType.is_gt, op1=mybir.AluOpType.max,
                )
                m2 = wpool.tile([B, B], F32)
                nc.vector.tensor_tensor(
                    out=m2, in0=m1, in1=ttp, op=mybir.AluOpType.max
                )
                final_mask = m2
            else:
                final_mask = wpool.tile([B, B], F32)
                nc.vector.tensor_scalar(
                    final_mask, s_sb, THRESHOLD, None, op0=mybir.AluOpType.is_gt
                )
            nc.sync.dma_start(mask_d, final_mask)

    nc.compile()
    return nc


def _prep_inputs(global_token, W1_global, b1_global, W2_global, b2_global):
    gt = np.ascontiguousarray(np.asarray(global_token, np.float32))
    W1 = np.asarray(W1_global, np.float32)
    b1 = np.asarray(b1_global, np.float32)
    W2 = np.asarray(W2_global, np.float32)
    b2 = np.asarray(b2_global, np.float32)

    gth = np.ascontiguousarray(gt.T.reshape(KD, 128, B).transpose(1, 0, 2))
    w1h = np.ascontiguousarray(W1.reshape(KD, 128, MLP).transpose(1, 0, 2))
    b1h = np.ascontiguousarray(b1.reshape(KM, 128).T)
    w2h = np.ascontiguousarray(W2.reshape(KM, 128, TD).transpose(1, 0, 2))
    b2r = np.ascontiguousarray(np.broadcast_to(b2[None, :], (B, TD)))
    return gth, w1h, b1h, w2h, b2r


def _make_in_maps(gth, w1h, b1h, w2h, b2r):
    in_maps = []
    for c in range(N_CORES):
        rsel = np.zeros((B, 128), np.float32)
        for p in range(128):
            rsel[c * RPC + p // 16, p] = 1.0
        selm = np.zeros((B, RPC), np.float32)
        for j in range(RPC):
            selm[c * RPC + j, j] = 1.0
        small = np.ascontiguousarray(
            np.concatenate([b2r, rsel, selm], axis=1)
        )
        in_maps.append({
            "gtT": gth, "w1": w1h, "b1": b1h, "w2": w2h, "small": small,
        })
    return in_maps


def kernel(local_features=None, global_token=None,
           W1_local=None, b1_local=None, W2_local=None, b2_local=None,
           W1_global=None, b1_global=None, W2_global=None, b2_global=None,
           k_nearest=10, **_unused):
    k = int(k_nearest)
    if k not in _cache:
        _cache[k] = _build(k)
    nc = _cache[k]

    gth, w1h, b1h, w2h, b2r = _prep_inputs(
        global_token, W1_global, b1_global, W2_global, b2_global
    )

    in_maps = _make_in_maps(gth, w1h, b1h, w2h, b2r)
    res = run_bass_kernel_spmd(nc, in_maps, core_ids=list(range(N_CORES)))
    feats = np.concatenate(
        [res.results[c]["feats"] for c in range(N_CORES)], axis=0
    )
    mask = res.results[0]["mask"]
    return feats, mask


# revision 27
# speedup vs baseline: 1.0468x; 1.0468x over previous
"""Trainium2 Bass kernel for nn_MegaLocMPS (retrieval_knn).

Reference computation:
    h      = relu(local_features @ W1_local + b1_local)            [B,N,mlp]
    local  = softmax(h @ W2_local + b2_local, axis=N).sum(axis=N)  [B,K*C]
    g      = relu(global_token @ W1_global + b1) @ W2_global + b2  [B,td]
    feats  = l2_normalize(concat([local, g]))                      [B,16640]
    sim    = feats @ feats.T ; mask = (sim > 0.7) | topk(sim, k) | diag

Key algebraic identity: softmax(x, axis=1).sum(axis=1) == 1 exactly, for any
input — so `local` is identically ones(B, 16384) and the whole local branch
(W1_local/W2_local, the 137-GMAC matmul) contributes only the constant 1.0.
Therefore:
    norm_b = sqrt(16384 + ||g_b||^2)
    feats  = [1/norm_b broadcast 16384 | g_b/norm_b]
    sim_ij = (16384 + g_i.g_j) * inv_i * inv_j
           = a_i . a_j  with a_i = [g_i*inv_i, 128*inv_i]   (257-dim)
The device kernel computes the global MLP, norms, sim, threshold, and an exact
top-k mask (top-8 `max` + `match_replace` + threshold at the k-th value).

Sharding: the tiny compute is replicated on all 8 cores; the large feats
output write ([64,16640] = 4.3 MB) is row-sharded 8 ways via per-core one-hot
selection matrices (SPMD-friendly: same program, different data).
"""

import os
import sys

for _p in ("/opt/trn_rl_repo", "/root/.axon_site/_ro/trn_rl_repo"):
    if os.path.isdir(_p) and _p not in sys.path:
        sys.path.append(_p)

import numpy as np

import concourse.bass as bass
import concourse.mybir as mybir
import concourse.tile as tile
from concourse import bacc
from concourse.bass_utils import run_bass_kernel_spmd
from concourse.masks import make_identity

F32 = mybir.dt.float32

B = 64          # batch (images)
D = 768         # input feature dim
MLP = 512       # hidden dim
TD = 256        # global token output dim
KC = 16384      # NUM_CLUSTERS * CLUSTER_DIM (all-ones part of feats)
KD = D // 128   # 6 contraction chunks for layer 1
KM = MLP // 128 # 4 chunks for layer 2
N_CORES = 8
RPC = B // N_CORES  # 8 output rows per core
THRESHOLD = 0.7

_cache = {}


def _build(k: int):
    """Build + compile the SPMD program (same on all cores)."""
    nc = bacc.Bacc("TRN2", debug=False, num_devices=N_CORES)

    # Inputs (host pre-tiled so every DMA is partition-contiguous).
    # small = [b2r (256) | rsel (128) | sel (8)] packed into one [64, 392].
    gtT_d = nc.dram_tensor("gtT", [128, KD, B], F32, kind="ExternalInput").ap()
    w1_d = nc.dram_tensor("w1", [128, KD, MLP], F32, kind="ExternalInput").ap()
    b1_d = nc.dram_tensor("b1", [128, KM], F32, kind="ExternalInput").ap()
    w2_d = nc.dram_tensor("w2", [128, KM, TD], F32, kind="ExternalInput").ap()
    small_d = nc.dram_tensor("small", [B, TD + 128 + RPC], F32,
                             kind="ExternalInput").ap()

    feats_d = nc.dram_tensor("feats", [RPC, KC + TD], F32, kind="ExternalOutput").ap()
    mask_d = nc.dram_tensor("mask", [B, B], F32, kind="ExternalOutput").ap()

    with tile.TileContext(nc) as tc:
        with (
            tc.tile_pool(name="const", bufs=1) as cpool,
            tc.tile_pool(name="work", bufs=1) as wpool,
            tc.tile_pool(name="psum", bufs=1, space="PSUM") as ppool,
        ):
            # ---- constants (gpsimd, off the DMA/compute path) -------------
            ident = cpool.tile([B, B], F32)
            make_identity(nc, ident)
            kc_const = cpool.tile([B, 1], F32)
            nc.gpsimd.memset(kc_const, float(KC))
            dumm = cpool.tile([1, 1], F32)
            nc.gpsimd.memset(dumm, 4.0)
            ones1 = cpool.tile([1, B], F32)
            nc.gpsimd.memset(ones1, 1.0)

            # ---- input DMAs, issue split across the two HWDGE engines -----
            # sync queue: gtT, w1 chunks {0,1}/{4,5}, b1
            # scalar queue: w1 chunks {2,3}, w2, packed small tile
            gtT_sb = cpool.tile([128, KD, B], F32)
            nc.sync.dma_start(gtT_sb, gtT_d)
            w1_sb = cpool.tile([128, KD, MLP], F32)
            nc.sync.dma_start(w1_sb[:, 0:1, :], w1_d[:, 0:1, :])
            nc.sync.dma_start(w1_sb[:, 1:3, :], w1_d[:, 1:3, :])
            nc.sync.dma_start(w1_sb[:, 3:6, :], w1_d[:, 3:6, :])
            b1_sb = cpool.tile([128, KM], F32)
            nc.sync.dma_start(b1_sb, b1_d)
            w2_sb = cpool.tile([128, KM, TD], F32)
            nc.sync.dma_start(w2_sb, w2_d)
            small_sb = cpool.tile([B, TD + 128 + RPC], F32)
            nc.sync.dma_start(small_sb, small_d)
            b2r_sb = small_sb[:, 0:TD]
            rsel_sb = small_sb[:, TD:TD + 128]
            sel_sb = small_sb[:, TD + 128:]

            # pre-warm both ACT function tables while the PE works
            # (Sqrt lives in a second table; loading it lazily would stall
            # the norm chain ~1.3us). Scheduled after scalar's DMA issues.
            dumo = cpool.tile([1, 1], F32)
            with tc.high_priority():
                nc.scalar.activation(dumo, dumm,
                                     mybir.ActivationFunctionType.Relu)
                nc.scalar.activation(dumo, dumm,
                                     mybir.ActivationFunctionType.Sqrt)

            # ---- layer 1: hT[m] = relu(W1[:,m].T @ gt.T + b1[m]) ----------
            # m processed in groups of two, k-outer within a group, so the
            # first matmul only needs w1 chunk 0 (overlaps the w1 DMA) and
            # group A's relu overlaps group B's matmuls.
            hT_sb = wpool.tile([128, KM, B], F32)
            hps = [
                ppool.tile([128, B], F32, tag=f"hps{m}", bufs=1,
                           name=f"hps{m}")
                for m in range(KM)
            ]
            for kk in range(KD):
                for m in range(KM):
                    nc.tensor.matmul(
                        hps[m],
                        lhsT=w1_sb[:, kk, 128 * m:128 * (m + 1)],
                        rhs=gtT_sb[:, kk, :],
                        start=(kk == 0),
                        stop=(kk == KD - 1),
                    )
            for m in range(KM):
                nc.scalar.activation(
                    hT_sb[:, m, :], hps[m],
                    mybir.ActivationFunctionType.Relu,
                    bias=b1_sb[:, m:m + 1],
                )

            # ---- layer 2: g = hT.T @ W2 + b2  [B, TD] ---------------------
            gps = ppool.tile([B, TD], F32, tag="acc", bufs=1)
            for m in range(KM):
                nc.tensor.matmul(
                    gps,
                    lhsT=hT_sb[:, m, :],
                    rhs=w2_sb[:, m, :],
                    start=(m == 0),
                    stop=(m == KM - 1),
                )
            g_sb = wpool.tile([B, TD], F32)
            nc.vector.tensor_add(g_sb, gps, b2r_sb)

            # ---- norms: norm = sqrt(16384 + sum(g^2)) ---------------------
            gsq = wpool.tile([B, TD], F32)
            ss = wpool.tile([B, 1], F32)
            nc.scalar.activation(
                gsq, g_sb, mybir.ActivationFunctionType.Square, accum_out=ss
            )
            norm = wpool.tile([B, 1], F32)
            nc.scalar.activation(
                norm, ss, mybir.ActivationFunctionType.Sqrt, bias=kc_const
            )
            inv = wpool.tile([B, 1], F32)
            nc.vector.reciprocal(inv, norm)
            inv128 = wpool.tile([B, 1], F32)
            nc.scalar.mul(inv128, inv, float(np.sqrt(KC)))

            gs = wpool.tile([B, TD], F32)  # g / norm  (the feats tail)
            nc.vector.tensor_scalar(
                gs, g_sb, inv, None, op0=mybir.AluOpType.mult
            )

            # ---- feats output (row shard) ---------------------------------
            # constant region: feats[b, :16384] = 1/norm_b
            irp = ppool.tile([128, 1], F32, tag="small", bufs=1)
            nc.tensor.matmul(irp, lhsT=rsel_sb, rhs=inv, start=True, stop=True)
            ir_sb = wpool.tile([128, 1], F32)
            nc.vector.tensor_copy(ir_sb, irp)
            fconst = wpool.tile([128, KC // 16], F32)
            nc.vector.tensor_copy(fconst, ir_sb.to_broadcast([128, KC // 16]))
            nc.sync.dma_start(feats_d[0:RPC // 2, 0:KC], fconst[0:64, :])
            nc.scalar.dma_start(feats_d[RPC // 2:, 0:KC], fconst[64:128, :])
            # g region: feats[b, 16384:] = g_b/norm_b for this core's rows
            gshp = ppool.tile([RPC, TD], F32, tag="small", bufs=1, name="gshp")
            nc.tensor.matmul(gshp, lhsT=sel_sb, rhs=gs, start=True, stop=True)
            gsh_sb = wpool.tile([RPC, TD], F32)
            nc.scalar.copy(gsh_sb, gshp)
            nc.sync.dma_start(feats_d[:, KC:], gsh_sb)

            # ---- similarity: sim = A A.T with a_i=[gs_i, 128*inv_i] -------
            gsT_sb = wpool.tile([128, 2, B], F32)
            for j in range(2):
                tp = ppool.tile([128, B], F32, tag="tp", bufs=2)
                nc.tensor.transpose(tp, gs[:, 128 * j:128 * (j + 1)], ident)
                nc.scalar.copy(gsT_sb[:, j, :], tp)
            ivp = ppool.tile([1, B], F32, tag="small", bufs=1, name="ivp")
            nc.tensor.transpose(ivp, inv128, ident)
            invT_sb = wpool.tile([1, B], F32)
            nc.scalar.copy(invT_sb, ivp)

            sps = ppool.tile([B, B], F32, tag="acc", bufs=1, name="sps")
            nc.tensor.matmul(sps, lhsT=gsT_sb[:, 0, :], rhs=gsT_sb[:, 0, :],
                             start=True, stop=False)
            nc.tensor.matmul(sps, lhsT=gsT_sb[:, 1, :], rhs=gsT_sb[:, 1, :],
                             start=False, stop=False)
            nc.tensor.matmul(sps, lhsT=invT_sb, rhs=invT_sb,
                             start=False, stop=True)

            # ---- mask: threshold | topk | topk.T --------------------------
            s_sb = wpool.tile([B, B], F32)
            nc.vector.tensor_copy(s_sb, sps)
            kk = min(k, B)
            if kk > 0:
                cur = s_sb
                rounds = (kk - 1) // 8
                for r in range(rounds):
                    mx = wpool.tile([B, 8], F32, tag="mx", bufs=2)
                    nc.vector.max(out=mx, in_=cur)
                    nxt = wpool.tile([B, B], F32, tag="scur", bufs=2)
                    nc.vector.match_replace(
                        out=nxt, in_to_replace=mx, in_values=cur,
                        imm_value=-10.0,
                    )
                    cur = nxt
                mxf = wpool.tile([B, 8], F32)
                nc.vector.max(out=mxf, in_=cur)
                pos = (kk - 1) % 8
                tmask = wpool.tile([B, B], F32)
                nc.vector.tensor_scalar(
                    tmask, s_sb, mxf[:, pos:pos + 1], None,
                    op0=mybir.AluOpType.is_ge,
                )
                ttp = ppool.tile([B, B], F32, tag="tp", bufs=2)
                nc.tensor.transpose(ttp, tmask, ident)
                # m1 = max(sim > 0.7, topk)   (one fused DVE op)
                m1 = wpool.tile([B, B], F32)
                nc.vector.scalar_tensor_tensor(
                    out=m1, in0=s_sb, scalar=THRESHOLD, in1=tmask,
                    op0=mybir.AluOpType.is_gt, op1=mybir.AluOpType.max,
                )
                m2 = wpool.tile([B, B], F32)
                nc.vector.tensor_tensor(
                    out=m2, in0=m1, in1=ttp, op=mybir.AluOpType.max
                )
                final_mask = m2
            else:
                final_mask = wpool.tile([B, B], F32)
                nc.vector.tensor_scalar(
                    final_mask, s_sb, THRESHOLD, None, op0=mybir.AluOpType.is_gt
                )
            nc.sync.dma_start(mask_d, final_mask)

    nc.compile()
    return nc


def _prep_inputs(global_token, W1_global, b1_global, W2_global, b2_global):
    gt = np.ascontiguousarray(np.asarray(global_token, np.float32))
    W1 = np.asarray(W1_global, np.float32)
    b1 = np.asarray(b1_global, np.float32)
    W2 = np.asarray(W2_global, np.float32)
    b2 = np.asarray(b2_global, np.float32)

    gth = np.ascontiguousarray(gt.T.reshape(KD, 128, B).transpose(1, 0, 2))
    w1h = np.ascontiguousarray(W1.reshape(KD, 128, MLP).transpose(1, 0, 2))
    b1h = np.ascontiguousarray(b1.reshape(KM, 128).T)
    w2h = np.ascontiguousarray(W2.reshape(KM, 128, TD).transpose(1, 0, 2))
    b2r = np.ascontiguousarray(np.broadcast_to(b2[None, :], (B, TD)))
    return gth, w1h, b1h, w2h, b2r


def _make_in_maps(gth, w1h, b1h, w2h, b2r):
    in_maps = []
    for c in range(N_CORES):
        rsel = np.zeros((B, 128), np.float32)
        for p in range(128):
            rsel[c * RPC + p // 16, p] = 1.0
        selm = np.zeros((B, RPC), np.float32)
        for j in range(RPC):
            selm[c * RPC + j, j] = 1.0
        small = np.ascontiguousarray(
            np.concatenate([b2r, rsel, selm], axis=1)
        )
        in_maps.append({
            "gtT": gth, "w1": w1h, "b1": b1h, "w2": w2h, "small": small,
        })
    return in_maps


def kernel(local_features=None, global_token=None,
           W1_local=None, b1_local=None, W2_local=None, b2_local=None,
           W1_global=None, b1_global=None, W2_global=None, b2_global=None,
           k_nearest=10, **_unused):
    k = int(k_nearest)
    if k not in _cache:
        _cache[k] = _build(k)
    nc = _cache[k]

    gth, w1h, b1h, w2h, b2r = _prep_inputs(
        global_token, W1_global, b1_global, W2_global, b2_global
    )

    in_maps = _make_in_maps(gth, w1h, b1h, w2h, b2r)
    res = run_bass_kernel_spmd(nc, in_maps, core_ids=list(range(N_CORES)))
    feats = np.concatenate(
        [res.results[c]["feats"] for c in range(N_CORES)], axis=0
    )
    mask = res.results[0]["mask"]
    return feats, mask


# revision 28
# speedup vs baseline: 1.0986x; 1.0495x over previous
"""Trainium2 Bass kernel for nn_MegaLocMPS (retrieval_knn).

Reference computation:
    h      = relu(local_features @ W1_local + b1_local)            [B,N,mlp]
    local  = softmax(h @ W2_local + b2_local, axis=N).sum(axis=N)  [B,K*C]
    g      = relu(global_token @ W1_global + b1) @ W2_global + b2  [B,td]
    feats  = l2_normalize(concat([local, g]))                      [B,16640]
    sim    = feats @ feats.T ; mask = (sim > 0.7) | topk(sim, k) | diag

Key algebraic identity: softmax(x, axis=1).sum(axis=1) == 1 exactly, for any
input — so `local` is identically ones(B, 16384) and the whole local branch
(W1_local/W2_local, the 137-GMAC matmul) contributes only the constant 1.0.
Therefore:
    norm_b = sqrt(16384 + ||g_b||^2)
    feats  = [1/norm_b broadcast 16384 | g_b/norm_b]
    sim_ij = (16384 + g_i.g_j) * inv_i * inv_j
           = a_i . a_j  with a_i = [g_i*inv_i, 128*inv_i]   (257-dim)
The device kernel computes the global MLP, norms, sim, threshold, and an exact
top-k mask (top-8 `max` + `match_replace` + threshold at the k-th value).

Sharding: the tiny compute is replicated on all 8 cores; the large feats
output write ([64,16640] = 4.3 MB) is row-sharded 8 ways via per-core one-hot
selection matrices (SPMD-friendly: same program, different data).
"""

import os
import sys

for _p in ("/opt/trn_rl_repo", "/root/.axon_site/_ro/trn_rl_repo"):
    if os.path.isdir(_p) and _p not in sys.path:
        sys.path.append(_p)

import numpy as np

import concourse.bass as bass
import concourse.mybir as mybir
import concourse.tile as tile
from concourse import bacc
from concourse.bass_utils import run_bass_kernel_spmd
from concourse.masks import make_identity

F32 = mybir.dt.float32

B = 64          # batch (images)
D = 768         # input feature dim
MLP = 512       # hidden dim
TD = 256        # global token output dim
KC = 16384      # NUM_CLUSTERS * CLUSTER_DIM (all-ones part of feats)
KD = D // 128   # 6 contraction chunks for layer 1
KM = MLP // 128 # 4 chunks for layer 2
N_CORES = 8
RPC = B // N_CORES  # 8 output rows per core
THRESHOLD = 0.7

_cache = {}


def _build(k: int):
    """Build + compile the SPMD program (same on all cores)."""
    nc = bacc.Bacc("TRN2", debug=False, num_devices=N_CORES)

    # Inputs (host pre-tiled so every DMA is partition-contiguous).
    # small = [b2r (256) | rsel (128) | sel (8)] packed into one [64, 392].
    gtT_d = nc.dram_tensor("gtT", [128, KD, B], F32, kind="ExternalInput").ap()
    w1_d = nc.dram_tensor("w1", [128, KD, MLP], F32, kind="ExternalInput").ap()
    b1_d = nc.dram_tensor("b1", [128, KM], F32, kind="ExternalInput").ap()
    w2_d = nc.dram_tensor("w2", [128, KM, TD], F32, kind="ExternalInput").ap()
    small_d = nc.dram_tensor("small", [B, TD + 128 + RPC], F32,
                             kind="ExternalInput").ap()

    feats_d = nc.dram_tensor("feats", [RPC, KC + TD], F32, kind="ExternalOutput").ap()
    mask_d = nc.dram_tensor("mask", [B, B], F32, kind="ExternalOutput").ap()

    with tile.TileContext(nc) as tc:
        with (
            tc.tile_pool(name="const", bufs=1) as cpool,
            tc.tile_pool(name="work", bufs=1) as wpool,
            tc.tile_pool(name="psum", bufs=1, space="PSUM") as ppool,
        ):
            # ---- constants (gpsimd, off the DMA/compute path) -------------
            ident = cpool.tile([B, B], F32)
            make_identity(nc, ident)
            kc_const = cpool.tile([B, 1], F32)
            nc.gpsimd.memset(kc_const, float(KC))
            dumm = cpool.tile([1, 1], F32)
            nc.gpsimd.memset(dumm, 4.0)
            ones1 = cpool.tile([1, B], F32)
            nc.gpsimd.memset(ones1, 1.0)

            # ---- input DMAs, issue split across the two HWDGE engines -----
            # sync queue: gtT, w1 chunks {0,1}/{4,5}, b1
            # scalar queue: w1 chunks {2,3}, w2, packed small tile
            gtT_sb = cpool.tile([128, KD, B], F32)
            nc.sync.dma_start(gtT_sb[:, 0:1, :], gtT_d[:, 0:1, :])
            w1_sb = cpool.tile([128, KD, MLP], F32)
            nc.sync.dma_start(w1_sb[:, 0:1, :], w1_d[:, 0:1, :])
            nc.sync.dma_start(gtT_sb[:, 1:6, :], gtT_d[:, 1:6, :])
            nc.sync.dma_start(w1_sb[:, 1:3, :], w1_d[:, 1:3, :])
            nc.sync.dma_start(w1_sb[:, 3:6, :], w1_d[:, 3:6, :])
            b1_sb = cpool.tile([128, KM], F32)
            nc.sync.dma_start(b1_sb, b1_d)
            w2_sb = cpool.tile([128, KM, TD], F32)
            nc.sync.dma_start(w2_sb, w2_d)
            small_sb = cpool.tile([B, TD + 128 + RPC], F32)
            nc.sync.dma_start(small_sb, small_d)
            b2r_sb = small_sb[:, 0:TD]
            rsel_sb = small_sb[:, TD:TD + 128]
            sel_sb = small_sb[:, TD + 128:]

            # pre-warm both ACT function tables while the PE works
            # (Sqrt lives in a second table; loading it lazily would stall
            # the norm chain ~1.3us). Scheduled after scalar's DMA issues.
            dumo = cpool.tile([1, 1], F32)
            with tc.high_priority():
                nc.scalar.activation(dumo, dumm,
                                     mybir.ActivationFunctionType.Relu)
                nc.scalar.activation(dumo, dumm,
                                     mybir.ActivationFunctionType.Sqrt)

            # ---- layer 1: hT[m] = relu(W1[:,m].T @ gt.T + b1[m]) ----------
            # m processed in groups of two, k-outer within a group, so the
            # first matmul only needs w1 chunk 0 (overlaps the w1 DMA) and
            # group A's relu overlaps group B's matmuls.
            hT_sb = wpool.tile([128, KM, B], F32)
            hps = [
                ppool.tile([128, B], F32, tag=f"hps{m}", bufs=1,
                           name=f"hps{m}")
                for m in range(KM)
            ]
            for kk in range(KD):
                for m in range(KM):
                    nc.tensor.matmul(
                        hps[m],
                        lhsT=w1_sb[:, kk, 128 * m:128 * (m + 1)],
                        rhs=gtT_sb[:, kk, :],
                        start=(kk == 0),
                        stop=(kk == KD - 1),
                    )
            for m in range(KM):
                nc.scalar.activation(
                    hT_sb[:, m, :], hps[m],
                    mybir.ActivationFunctionType.Relu,
                    bias=b1_sb[:, m:m + 1],
                )

            # ---- layer 2: g = hT.T @ W2 + b2  [B, TD] ---------------------
            gps = ppool.tile([B, TD], F32, tag="acc", bufs=1)
            for m in range(KM):
                nc.tensor.matmul(
                    gps,
                    lhsT=hT_sb[:, m, :],
                    rhs=w2_sb[:, m, :],
                    start=(m == 0),
                    stop=(m == KM - 1),
                )
            g_sb = wpool.tile([B, TD], F32)
            nc.vector.tensor_add(g_sb, gps, b2r_sb)

            # ---- norms: norm = sqrt(16384 + sum(g^2)) ---------------------
            gsq = wpool.tile([B, TD], F32)
            ss = wpool.tile([B, 1], F32)
            nc.scalar.activation(
                gsq, g_sb, mybir.ActivationFunctionType.Square, accum_out=ss
            )
            norm = wpool.tile([B, 1], F32)
            nc.scalar.activation(
                norm, ss, mybir.ActivationFunctionType.Sqrt, bias=kc_const
            )
            inv = wpool.tile([B, 1], F32)
            nc.vector.reciprocal(inv, norm)
            inv128 = wpool.tile([B, 1], F32)
            nc.scalar.mul(inv128, inv, float(np.sqrt(KC)))

            gs = wpool.tile([B, TD], F32)  # g / norm  (the feats tail)
            nc.vector.tensor_scalar(
                gs, g_sb, inv, None, op0=mybir.AluOpType.mult
            )

            # ---- feats output (row shard) ---------------------------------
            # constant region: feats[b, :16384] = 1/norm_b
            irp = ppool.tile([128, 1], F32, tag="small", bufs=1)
            nc.tensor.matmul(irp, lhsT=rsel_sb, rhs=inv, start=True, stop=True)
            ir_sb = wpool.tile([128, 1], F32)
            nc.vector.tensor_copy(ir_sb, irp)
            fconst = wpool.tile([128, KC // 16], F32)
            nc.vector.tensor_copy(fconst, ir_sb.to_broadcast([128, KC // 16]))
            nc.sync.dma_start(feats_d[:, 0:KC], fconst)
            # g region: feats[b, 16384:] = g_b/norm_b for this core's rows
            gshp = ppool.tile([RPC, TD], F32, tag="small", bufs=1, name="gshp")
            nc.tensor.matmul(gshp, lhsT=sel_sb, rhs=gs, start=True, stop=True)
            gsh_sb = wpool.tile([RPC, TD], F32)
            nc.scalar.copy(gsh_sb, gshp)
            nc.sync.dma_start(feats_d[:, KC:], gsh_sb)

            # ---- similarity: sim = A A.T with a_i=[gs_i, 128*inv_i] -------
            gsT_sb = wpool.tile([128, 2, B], F32)
            for j in range(2):
                tp = ppool.tile([128, B], F32, tag="tp", bufs=2)
                nc.tensor.transpose(tp, gs[:, 128 * j:128 * (j + 1)], ident)
                nc.scalar.copy(gsT_sb[:, j, :], tp)
            ivp = ppool.tile([1, B], F32, tag="small", bufs=1, name="ivp")
            nc.tensor.transpose(ivp, inv128, ident)
            invT_sb = wpool.tile([1, B], F32)
            nc.scalar.copy(invT_sb, ivp)

            sps = ppool.tile([B, B], F32, tag="acc", bufs=1, name="sps")
            nc.tensor.matmul(sps, lhsT=gsT_sb[:, 0, :], rhs=gsT_sb[:, 0, :],
                             start=True, stop=False)
            nc.tensor.matmul(sps, lhsT=gsT_sb[:, 1, :], rhs=gsT_sb[:, 1, :],
                             start=False, stop=False)
            nc.tensor.matmul(sps, lhsT=invT_sb, rhs=invT_sb,
                             start=False, stop=True)

            # ---- mask: threshold | topk | topk.T --------------------------
            s_sb = wpool.tile([B, B], F32)
            nc.vector.tensor_copy(s_sb, sps)
            kk = min(k, B)
            if kk > 0:
                cur = s_sb
                rounds = (kk - 1) // 8
                for r in range(rounds):
                    mx = wpool.tile([B, 8], F32, tag="mx", bufs=2)
                    nc.vector.max(out=mx, in_=cur)
                    nxt = wpool.tile([B, B], F32, tag="scur", bufs=2)
                    nc.vector.match_replace(
                        out=nxt, in_to_replace=mx, in_values=cur,
                        imm_value=-10.0,
                    )
                    cur = nxt
                mxf = wpool.tile([B, 8], F32)
                nc.vector.max(out=mxf, in_=cur)
                pos = (kk - 1) % 8
                tmask = wpool.tile([B, B], F32)
                nc.vector.tensor_scalar(
                    tmask, s_sb, mxf[:, pos:pos + 1], None,
                    op0=mybir.AluOpType.is_ge,
                )
                ttp = ppool.tile([B, B], F32, tag="tp", bufs=2)
                nc.tensor.transpose(ttp, tmask, ident)
                # m1 = max(sim > 0.7, topk)   (one fused DVE op)
                m1 = wpool.tile([B, B], F32)
                nc.vector.scalar_tensor_tensor(
                    out=m1, in0=s_sb, scalar=THRESHOLD, in1=tmask,
                    op0=mybir.AluOpType.is_gt, op1=mybir.AluOpType.max,
                )
                m2 = wpool.tile([B, B], F32)
                nc.vector.tensor_tensor(
                    out=m2, in0=m1, in1=ttp, op=mybir.AluOpType.max
                )
                final_mask = m2
            else:
                final_mask = wpool.tile([B, B], F32)
                nc.vector.tensor_scalar(
                    final_mask, s_sb, THRESHOLD, None, op0=mybir.AluOpType.is_gt
                )
            nc.sync.dma_start(mask_d, final_mask)

    nc.compile()
    return nc


def _prep_inputs(global_token, W1_global, b1_global, W2_global, b2_global):
    gt = np.ascontiguousarray(np.asarray(global_token, np.float32))
    W1 = np.asarray(W1_global, np.float32)
    b1 = np.asarray(b1_global, np.float32)
    W2 = np.asarray(W2_global, np.float32)
    b2 = np.asarray(b2_global, np.float32)

    gth = np.ascontiguousarray(gt.T.reshape(KD, 128, B).transpose(1, 0, 2))
    w1h = np.ascontiguousarray(W1.reshape(KD, 128, MLP).transpose(1, 0, 2))
    b1h = np.ascontiguousarray(b1.reshape(KM, 128).T)
    w2h = np.ascontiguousarray(W2.reshape(KM, 128, TD).transpose(1, 0, 2))
    b2r = np.ascontiguousarray(np.broadcast_to(b2[None, :], (B, TD)))
    return gth, w1h, b1h, w2h, b2r


def _make_in_maps(gth, w1h, b1h, w2h, b2r):
    in_maps = []
    for c in range(N_CORES):
        rsel = np.zeros((B, 128), np.float32)
        for p in range(128):
            rsel[c * RPC + p // 16, p] = 1.0
        selm = np.zeros((B, RPC), np.float32)
        for j in range(RPC):
            selm[c * RPC + j, j] = 1.0
        small = np.ascontiguousarray(
            np.concatenate([b2r, rsel, selm], axis=1)
        )
        in_maps.append({
            "gtT": gth, "w1": w1h, "b1": b1h, "w2": w2h, "small": small,
        })
    return in_maps


def kernel(local_features=None, global_token=None,
           W1_local=None, b1_local=None, W2_local=None, b2_local=None,
           W1_global=None, b1_global=None, W2_global=None, b2_global=None,
           k_nearest=10, **_unused):
    k = int(k_nearest)
    if k not in _cache:
        _cache[k] = _build(k)
    nc = _cache[k]

    gth, w1h, b1h, w2h, b2r = _prep_inputs(
        global_token, W1_global, b1_global, W2_global, b2_global
    )

    in_maps = _make_in_maps(gth, w1h, b1h, w2h, b2r)
    res = run_bass_kernel_spmd(nc, in_maps, core_ids=list(range(N_CORES)))
    feats = np.concatenate(
        [res.results[c]["feats"] for c in range(N_CORES)], axis=0
    )
    mask = res.results[0]["mask"]
    return feats, mask


# revision 30
# speedup vs baseline: 1.1205x; 1.0199x over previous
"""Trainium2 Bass kernel for nn_MegaLocMPS (retrieval_knn).

Reference computation:
    h      = relu(local_features @ W1_local + b1_local)            [B,N,mlp]
    local  = softmax(h @ W2_local + b2_local, axis=N).sum(axis=N)  [B,K*C]
    g      = relu(global_token @ W1_global + b1) @ W2_global + b2  [B,td]
    feats  = l2_normalize(concat([local, g]))                      [B,16640]
    sim    = feats @ feats.T ; mask = (sim > 0.7) | topk(sim, k) | diag

Key algebraic identity: softmax(x, axis=1).sum(axis=1) == 1 exactly, for any
input — so `local` is identically ones(B, 16384) and the whole local branch
(W1_local/W2_local, the 137-GMAC matmul) contributes only the constant 1.0.
Therefore:
    norm_b = sqrt(16384 + ||g_b||^2)
    feats  = [1/norm_b broadcast 16384 | g_b/norm_b]
    sim_ij = (16384 + g_i.g_j) * inv_i * inv_j
           = a_i . a_j  with a_i = [g_i*inv_i, 128*inv_i]   (257-dim)
The device kernel computes the global MLP, norms, sim, threshold, and an exact
top-k mask (top-8 `max` + `match_replace` + threshold at the k-th value).

Sharding: the tiny compute is replicated on all 8 cores; the large feats
output write ([64,16640] = 4.3 MB) is row-sharded 8 ways via per-core one-hot
selection matrices (SPMD-friendly: same program, different data).
"""

import os
import sys

for _p in ("/opt/trn_rl_repo", "/root/.axon_site/_ro/trn_rl_repo"):
    if os.path.isdir(_p) and _p not in sys.path:
        sys.path.append(_p)

import numpy as np

import concourse.bass as bass
import concourse.mybir as mybir
import concourse.tile as tile
from concourse import bacc
from concourse.bass_utils import run_bass_kernel_spmd
from concourse.masks import make_identity

F32 = mybir.dt.float32

B = 64          # batch (images)
D = 768         # input feature dim
MLP = 512       # hidden dim
TD = 256        # global token output dim
KC = 16384      # NUM_CLUSTERS * CLUSTER_DIM (all-ones part of feats)
KD = D // 128   # 6 contraction chunks for layer 1
KM = MLP // 128 # 4 chunks for layer 2
N_CORES = 8
RPC = B // N_CORES  # 8 output rows per core
THRESHOLD = 0.7

_cache = {}


def _build(k: int):
    """Build + compile the SPMD program (same on all cores)."""
    nc = bacc.Bacc("TRN2", debug=False, num_devices=N_CORES)

    # Inputs (host pre-tiled so every DMA is partition-contiguous).
    # small = [b2r (256) | rsel (128) | sel (8)] packed into one [64, 392].
    gtT_d = nc.dram_tensor("gtT", [128, KD, B], F32, kind="ExternalInput").ap()
    w1_d = nc.dram_tensor("w1", [128, KD, MLP], F32, kind="ExternalInput").ap()
    b1_d = nc.dram_tensor("b1", [128, KM], F32, kind="ExternalInput").ap()
    w2_d = nc.dram_tensor("w2", [128, KM, TD], F32, kind="ExternalInput").ap()
    small_d = nc.dram_tensor("small", [B, TD + 128 + RPC], F32,
                             kind="ExternalInput").ap()

    feats_d = nc.dram_tensor("feats", [RPC, KC + TD], F32, kind="ExternalOutput").ap()
    mask_d = nc.dram_tensor("mask", [B, B], F32, kind="ExternalOutput").ap()

    with tile.TileContext(nc) as tc:
        with (
            tc.tile_pool(name="const", bufs=1) as cpool,
            tc.tile_pool(name="work", bufs=1) as wpool,
            tc.tile_pool(name="psum", bufs=1, space="PSUM") as ppool,
        ):
            # ---- constants (gpsimd, off the DMA/compute path) -------------
            ident = cpool.tile([B, B], F32)
            make_identity(nc, ident)
            kc_const = cpool.tile([B, 1], F32)
            nc.gpsimd.memset(kc_const, float(KC))
            dumm = cpool.tile([1, 1], F32)
            nc.gpsimd.memset(dumm, 4.0)
            ones1 = cpool.tile([1, B], F32)
            nc.gpsimd.memset(ones1, 1.0)

            # ---- input DMAs, issue split across the two HWDGE engines -----
            # sync queue: gtT, w1 chunks {0,1}/{4,5}, b1
            # scalar queue: w1 chunks {2,3}, w2, packed small tile
            gtT_sb = cpool.tile([128, KD, B], F32)
            nc.sync.dma_start(gtT_sb[:, 0:1, :], gtT_d[:, 0:1, :])
            w1_sb = cpool.tile([128, KD, MLP], F32)
            nc.sync.dma_start(w1_sb[:, 0:1, :], w1_d[:, 0:1, :])
            nc.sync.dma_start(gtT_sb[:, 1:6, :], gtT_d[:, 1:6, :])
            nc.sync.dma_start(w1_sb[:, 1:3, :], w1_d[:, 1:3, :])
            nc.sync.dma_start(w1_sb[:, 3:6, :], w1_d[:, 3:6, :])
            b1_sb = cpool.tile([128, KM], F32)
            nc.sync.dma_start(b1_sb, b1_d)
            w2_sb = cpool.tile([128, KM, TD], F32)
            nc.sync.dma_start(w2_sb, w2_d)
            small_sb = cpool.tile([B, TD + 128 + RPC], F32)
            nc.sync.dma_start(small_sb, small_d)
            b2r_sb = small_sb[:, 0:TD]
            rsel_sb = small_sb[:, TD:TD + 128]
            sel_sb = small_sb[:, TD + 128:]

            # pre-warm both ACT function tables while the PE works
            # (Sqrt lives in a second table; loading it lazily would stall
            # the norm chain ~1.3us). Scheduled after scalar's DMA issues.
            dumo = cpool.tile([1, 1], F32)
            with tc.high_priority():
                nc.scalar.activation(dumo, dumm,
                                     mybir.ActivationFunctionType.Relu)
                nc.scalar.activation(dumo, dumm,
                                     mybir.ActivationFunctionType.Sqrt)

            # ---- layer 1: hT[m] = relu(W1[:,m].T @ gt.T + b1[m]) ----------
            # m processed in groups of two, k-outer within a group, so the
            # first matmul only needs w1 chunk 0 (overlaps the w1 DMA) and
            # group A's relu overlaps group B's matmuls.
            hT_sb = wpool.tile([128, KM, B], F32)
            hps = [
                ppool.tile([128, B], F32, tag=f"hps{m}", bufs=1,
                           name=f"hps{m}")
                for m in range(KM)
            ]
            for kk in range(KD):
                for m in range(KM):
                    nc.tensor.matmul(
                        hps[m],
                        lhsT=w1_sb[:, kk, 128 * m:128 * (m + 1)],
                        rhs=gtT_sb[:, kk, :],
                        start=(kk == 0),
                        stop=(kk == KD - 1),
                    )
            for m in range(KM):
                nc.scalar.activation(
                    hT_sb[:, m, :], hps[m],
                    mybir.ActivationFunctionType.Relu,
                    bias=b1_sb[:, m:m + 1],
                )

            # ---- layer 2: g = hT.T @ W2 + b2  [B, TD] ---------------------
            gps = ppool.tile([B, TD], F32, tag="acc", bufs=1)
            for m in range(KM):
                nc.tensor.matmul(
                    gps,
                    lhsT=hT_sb[:, m, :],
                    rhs=w2_sb[:, m, :],
                    start=(m == 0),
                    stop=(m == KM - 1),
                )
            g_sb = wpool.tile([B, TD], F32)
            nc.vector.tensor_add(g_sb, gps, b2r_sb)

            # ---- norms: norm = sqrt(16384 + sum(g^2)) ---------------------
            gsq = wpool.tile([B, TD], F32)
            ss = wpool.tile([B, 1], F32)
            nc.scalar.activation(
                gsq, g_sb, mybir.ActivationFunctionType.Square, accum_out=ss
            )
            norm = wpool.tile([B, 1], F32)
            nc.scalar.activation(
                norm, ss, mybir.ActivationFunctionType.Sqrt, bias=kc_const
            )
            inv = wpool.tile([B, 1], F32)
            nc.vector.reciprocal(inv, norm)
            inv128 = wpool.tile([B, 1], F32)
            nc.scalar.mul(inv128, inv, float(np.sqrt(KC)))

            gs = wpool.tile([B, TD], F32)  # g / norm  (the feats tail)
            nc.vector.tensor_scalar(
                gs, g_sb, inv, None, op0=mybir.AluOpType.mult
            )

            # ---- feats output (row shard) ---------------------------------
            # constant region: feats[b, :16384] = 1/norm_b
            irp = ppool.tile([128, 1], F32, tag="small", bufs=1)
            nc.tensor.matmul(irp, lhsT=rsel_sb, rhs=inv, start=True, stop=True)
            ir_sb = wpool.tile([128, 1], F32)
            nc.vector.tensor_copy(ir_sb, irp)
            fconst = wpool.tile([128, KC // 16], F32)
            nc.vector.tensor_copy(fconst, ir_sb.to_broadcast([128, KC // 16]))
            nc.sync.dma_start(feats_d[:, 0:KC], fconst)
            # g region: feats[b, 16384:] = g_b/norm_b for this core's rows
            gshp = ppool.tile([RPC, TD], F32, tag="small", bufs=1, name="gshp")
            nc.tensor.matmul(gshp, lhsT=sel_sb, rhs=gs, start=True, stop=True)
            gsh_sb = wpool.tile([RPC, TD], F32)
            nc.scalar.copy(gsh_sb, gshp)
            nc.sync.dma_start(feats_d[:, KC:], gsh_sb)

            # ---- similarity: sim = A A.T with a_i=[gs_i, 128*inv_i] -------
            gsT_sb = wpool.tile([128, 2, B], F32)
            for j in range(2):
                tp = ppool.tile([128, B], F32, tag="tp", bufs=2)
                nc.tensor.transpose(tp, gs[:, 128 * j:128 * (j + 1)], ident)
                nc.scalar.copy(gsT_sb[:, j, :], tp)
            ivp = ppool.tile([1, B], F32, tag="small", bufs=1, name="ivp")
            nc.tensor.transpose(ivp, inv128, ident)
            invT_sb = wpool.tile([1, B], F32)
            nc.scalar.copy(invT_sb, ivp)

            sps = ppool.tile([B, B], F32, tag="acc", bufs=1, name="sps")
            nc.tensor.matmul(sps, lhsT=gsT_sb[:, 0, :], rhs=gsT_sb[:, 0, :],
                             start=True, stop=False)
            nc.tensor.matmul(sps, lhsT=gsT_sb[:, 1, :], rhs=gsT_sb[:, 1, :],
                             start=False, stop=False)
            nc.tensor.matmul(sps, lhsT=invT_sb, rhs=invT_sb,
                             start=False, stop=True)

            # ---- mask: threshold | topk | topk.T --------------------------
            s_sb = wpool.tile([B, B], F32)
            nc.vector.tensor_copy(s_sb, sps)
            kk = min(k, B)
            if kk > 0:
                cur = s_sb
                rounds = (kk - 1) // 8
                for r in range(rounds):
                    mx = wpool.tile([B, 8], F32, tag="mx", bufs=2)
                    nc.vector.max(out=mx, in_=cur)
                    nxt = wpool.tile([B, B], F32, tag="scur", bufs=2)
                    nc.vector.match_replace(
                        out=nxt, in_to_replace=mx, in_values=cur,
                        imm_value=-10.0,
                    )
                    cur = nxt
                mxf = wpool.tile([B, 8], F32)
                nc.vector.max(out=mxf, in_=cur)
                pos = (kk - 1) % 8
                tmask = wpool.tile([B, B], F32)
                nc.vector.tensor_scalar(
                    tmask, s_sb, mxf[:, pos:pos + 1], None,
                    op0=mybir.AluOpType.is_ge,
                )
                ttp = ppool.tile([B, B], F32, tag="tp", bufs=2)
                nc.tensor.transpose(ttp, tmask, ident)
                # m1 = max(sim > 0.7, topk)   (one fused DVE op)
                m1 = wpool.tile([B, B], F32)
                nc.vector.scalar_tensor_tensor(
                    out=m1, in0=s_sb, scalar=THRESHOLD, in1=tmask,
                    op0=mybir.AluOpType.is_gt, op1=mybir.AluOpType.max,
                )
                m2 = wpool.tile([B, B], F32)
                nc.vector.tensor_tensor(
                    out=m2, in0=m1, in1=ttp, op=mybir.AluOpType.max
                )
                final_mask = m2
            else:
                final_mask = wpool.tile([B, B], F32)
                nc.vector.tensor_scalar(
                    final_mask, s_sb, THRESHOLD, None, op0=mybir.AluOpType.is_gt
                )
            nc.sync.dma_start(mask_d, final_mask)

    nc.compile()
    return nc


def _prep_inputs(global_token, W1_global, b1_global, W2_global, b2_global):
    gt = np.ascontiguousarray(np.asarray(global_token, np.float32))
    W1 = np.asarray(W1_global, np.float32)
    b1 = np.asarray(b1_global, np.float32)
    W2 = np.asarray(W2_global, np.float32)
    b2 = np.asarray(b2_global, np.float32)

    gth = np.ascontiguousarray(gt.T.reshape(KD, 128, B).transpose(1, 0, 2))
    w1h = np.ascontiguousarray(W1.reshape(KD, 128, MLP).transpose(1, 0, 2))
    b1h = np.ascontiguousarray(b1.reshape(KM, 128).T)
    w2h = np.ascontiguousarray(W2.reshape(KM, 128, TD).transpose(1, 0, 2))
    b2r = np.ascontiguousarray(np.broadcast_to(b2[None, :], (B, TD)))
    return gth, w1h, b1h, w2h, b2r


def _make_in_maps(gth, w1h, b1h, w2h, b2r):
    in_maps = []
    for c in range(N_CORES):
        rsel = np.zeros((B, 128), np.float32)
        for p in range(128):
            rsel[c * RPC + p // 16, p] = 1.0
        selm = np.zeros((B, RPC), np.float32)
        for j in range(RPC):
            selm[c * RPC + j, j] = 1.0
        small = np.ascontiguousarray(
            np.concatenate([b2r, rsel, selm], axis=1)
        )
        in_maps.append({
            "gtT": gth, "w1": w1h, "b1": b1h, "w2": w2h, "small": small,
        })
    return in_maps


def kernel(local_features=None, global_token=None,
           W1_local=None, b1_local=None, W2_local=None, b2_local=None,
           W1_global=None, b1_global=None, W2_global=None, b2_global=None,
           k_nearest=10, **_unused):
    k = int(k_nearest)
    if k not in _cache:
        _cache[k] = _build(k)
    nc = _cache[k]

    gth, w1h, b1h, w2h, b2r = _prep_inputs(
        global_token, W1_global, b1_global, W2_global, b2_global
    )

    in_maps = _make_in_maps(gth, w1h, b1h, w2h, b2r)
    res = run_bass_kernel_spmd(nc, in_maps, core_ids=list(range(N_CORES)))
    feats = np.concatenate(
        [res.results[c]["feats"] for c in range(N_CORES)], axis=0
    )
    mask = res.results[0]["mask"]
    return feats, mask


# revision 31
# speedup vs baseline: 1.1360x; 1.0138x over previous
"""Trainium2 Bass kernel for nn_MegaLocMPS (retrieval_knn).

Reference computation:
    h      = relu(local_features @ W1_local + b1_local)            [B,N,mlp]
    local  = softmax(h @ W2_local + b2_local, axis=N).sum(axis=N)  [B,K*C]
    g      = relu(global_token @ W1_global + b1) @ W2_global + b2  [B,td]
    feats  = l2_normalize(concat([local, g]))                      [B,16640]
    sim    = feats @ feats.T ; mask = (sim > 0.7) | topk(sim, k) | diag

Key algebraic identity: softmax(x, axis=1).sum(axis=1) == 1 exactly, for any
input — so `local` is identically ones(B, 16384) and the whole local branch
(W1_local/W2_local, the 137-GMAC matmul) contributes only the constant 1.0.
Therefore:
    norm_b = sqrt(16384 + ||g_b||^2)
    feats  = [1/norm_b broadcast 16384 | g_b/norm_b]
    sim_ij = (16384 + g_i.g_j) * inv_i * inv_j
           = a_i . a_j  with a_i = [g_i*inv_i, 128*inv_i]   (257-dim)
The device kernel computes the global MLP, norms, sim, threshold, and an exact
top-k mask (top-8 `max` + `match_replace` + threshold at the k-th value).

Sharding: the tiny compute is replicated on all 8 cores; the large feats
output write ([64,16640] = 4.3 MB) is row-sharded 8 ways via per-core one-hot
selection matrices (SPMD-friendly: same program, different data).
"""

import os
import sys

for _p in ("/opt/trn_rl_repo", "/root/.axon_site/_ro/trn_rl_repo"):
    if os.path.isdir(_p) and _p not in sys.path:
        sys.path.append(_p)

import numpy as np

import concourse.bass as bass
import concourse.mybir as mybir
import concourse.tile as tile
from concourse import bacc
from concourse.bass_utils import run_bass_kernel_spmd
from concourse.masks import make_identity

F32 = mybir.dt.float32

B = 64          # batch (images)
D = 768         # input feature dim
MLP = 512       # hidden dim
TD = 256        # global token output dim
KC = 16384      # NUM_CLUSTERS * CLUSTER_DIM (all-ones part of feats)
KD = D // 128   # 6 contraction chunks for layer 1
KM = MLP // 128 # 4 chunks for layer 2
N_CORES = 8
RPC = B // N_CORES  # 8 output rows per core
THRESHOLD = 0.7

_cache = {}


def _build(k: int):
    """Build + compile the SPMD program (same on all cores)."""
    nc = bacc.Bacc("TRN2", debug=False, num_devices=N_CORES)

    # Inputs (host pre-tiled so every DMA is partition-contiguous).
    # small = [b2r (256) | rsel (128) | sel (8)] packed into one [64, 392].
    gtT_d = nc.dram_tensor("gtT", [128, KD, B], F32, kind="ExternalInput").ap()
    w1_d = nc.dram_tensor("w1", [128, KD, MLP], F32, kind="ExternalInput").ap()
    b1_d = nc.dram_tensor("b1", [128, KM], F32, kind="ExternalInput").ap()
    w2_d = nc.dram_tensor("w2", [128, KM, TD], F32, kind="ExternalInput").ap()
    small_d = nc.dram_tensor("small", [B, TD + 128 + RPC], F32,
                             kind="ExternalInput").ap()

    feats_d = nc.dram_tensor("feats", [RPC, KC + TD], F32, kind="ExternalOutput").ap()
    mask_d = nc.dram_tensor("mask", [B, B], F32, kind="ExternalOutput").ap()

    with tile.TileContext(nc) as tc:
        with (
            tc.tile_pool(name="const", bufs=1) as cpool,
            tc.tile_pool(name="work", bufs=1) as wpool,
            tc.tile_pool(name="psum", bufs=1, space="PSUM") as ppool,
        ):
            # ---- constants (gpsimd, off the DMA/compute path) -------------
            ident = cpool.tile([B, B], F32)
            make_identity(nc, ident)
            kc_const = cpool.tile([B, 1], F32)
            nc.gpsimd.memset(kc_const, float(KC))
            dumm = cpool.tile([1, 1], F32)
            nc.gpsimd.memset(dumm, 4.0)
            ones1 = cpool.tile([1, B], F32)
            nc.gpsimd.memset(ones1, 1.0)

            # ---- input DMAs, issue split across the two HWDGE engines -----
            # sync queue: gtT, w1 chunks {0,1}/{4,5}, b1
            # scalar queue: w1 chunks {2,3}, w2, packed small tile
            gtT_sb = cpool.tile([128, KD, B], F32)
            nc.sync.dma_start(gtT_sb[:, 0:1, :], gtT_d[:, 0:1, :])
            w1_sb = cpool.tile([128, KD, MLP], F32)
            nc.sync.dma_start(w1_sb[:, 0:1, :], w1_d[:, 0:1, :])
            nc.sync.dma_start(gtT_sb[:, 1:6, :], gtT_d[:, 1:6, :])
            nc.sync.dma_start(w1_sb[:, 1:3, :], w1_d[:, 1:3, :])
            nc.sync.dma_start(w1_sb[:, 3:6, :], w1_d[:, 3:6, :])
            b1_sb = cpool.tile([128, KM], F32)
            nc.sync.dma_start(b1_sb, b1_d)
            w2_sb = cpool.tile([128, KM, TD], F32)
            nc.sync.dma_start(w2_sb, w2_d)
            small_sb = cpool.tile([B, TD + 128 + RPC], F32)
            nc.sync.dma_start(small_sb, small_d)
            b2r_sb = small_sb[:, 0:TD]
            rsel_sb = small_sb[:, TD:TD + 128]
            sel_sb = small_sb[:, TD + 128:]

            # pre-warm both ACT function tables while the PE works
            # (Sqrt lives in a second table; loading it lazily would stall
            # the norm chain ~1.3us). Scheduled after scalar's DMA issues.
            dumo = cpool.tile([1, 1], F32)
            with tc.high_priority():
                nc.scalar.activation(dumo, dumm,
                                     mybir.ActivationFunctionType.Relu)
                nc.scalar.activation(dumo, dumm,
                                     mybir.ActivationFunctionType.Sqrt)

            # ---- layer 1: hT[m] = relu(W1[:,m].T @ gt.T + b1[m]) ----------
            # m processed in groups of two, k-outer within a group, so the
            # first matmul only needs w1 chunk 0 (overlaps the w1 DMA) and
            # group A's relu overlaps group B's matmuls.
            hT_sb = wpool.tile([128, KM, B], F32)
            hps = [
                ppool.tile([128, B], F32, tag=f"hps{m}", bufs=1,
                           name=f"hps{m}")
                for m in range(KM)
            ]
            for kk in range(KD):
                for m in range(KM):
                    nc.tensor.matmul(
                        hps[m],
                        lhsT=w1_sb[:, kk, 128 * m:128 * (m + 1)],
                        rhs=gtT_sb[:, kk, :],
                        start=(kk == 0),
                        stop=(kk == KD - 1),
                    )
            for m in range(KM):
                nc.scalar.activation(
                    hT_sb[:, m, :], hps[m],
                    mybir.ActivationFunctionType.Relu,
                    bias=b1_sb[:, m:m + 1],
                )

            # ---- layer 2: g = hT.T @ W2 + b2  [B, TD] ---------------------
            gps = ppool.tile([B, TD], F32, tag="acc", bufs=1)
            for m in range(KM):
                nc.tensor.matmul(
                    gps,
                    lhsT=hT_sb[:, m, :],
                    rhs=w2_sb[:, m, :],
                    start=(m == 0),
                    stop=(m == KM - 1),
                )
            g_sb = wpool.tile([B, TD], F32)
            nc.vector.tensor_add(g_sb, gps, b2r_sb)

            # ---- norms: norm = sqrt(16384 + sum(g^2)) ---------------------
            gsq = wpool.tile([B, TD], F32)
            ss = wpool.tile([B, 1], F32)
            nc.scalar.activation(
                gsq, g_sb, mybir.ActivationFunctionType.Square, accum_out=ss
            )
            norm = wpool.tile([B, 1], F32)
            nc.scalar.activation(
                norm, ss, mybir.ActivationFunctionType.Sqrt, bias=kc_const
            )
            inv = wpool.tile([B, 1], F32)
            nc.vector.reciprocal(inv, norm)
            inv128 = wpool.tile([B, 1], F32)
            nc.scalar.mul(inv128, inv, float(np.sqrt(KC)))

            gs = wpool.tile([B, TD], F32)  # g / norm  (the feats tail)
            nc.vector.tensor_scalar(
                gs, g_sb, inv, None, op0=mybir.AluOpType.mult
            )

            # ---- feats output (row shard) ---------------------------------
            # constant region: feats[b, :16384] = 1/norm_b
            irp = ppool.tile([128, 1], F32, tag="small", bufs=1)
            nc.tensor.matmul(irp, lhsT=rsel_sb, rhs=inv, start=True, stop=True)
            ir_sb = wpool.tile([128, 1], F32)
            nc.vector.tensor_copy(ir_sb, irp)
            fconst = wpool.tile([128, KC // 16], F32)
            nc.vector.tensor_copy(fconst, ir_sb.to_broadcast([128, KC // 16]))
            nc.sync.dma_start(feats_d[:, 0:KC], fconst)

            # ---- similarity: sim = A A.T with a_i=[gs_i, 128*inv_i] -------
            gsT_sb = wpool.tile([128, 2, B], F32)
            for j in range(2):
                tp = ppool.tile([128, B], F32, tag="tp", bufs=2)
                nc.tensor.transpose(tp, gs[:, 128 * j:128 * (j + 1)], ident)
                nc.scalar.copy(gsT_sb[:, j, :], tp)
            ivp = ppool.tile([1, B], F32, tag="small", bufs=1, name="ivp")
            nc.tensor.transpose(ivp, inv128, ident)
            invT_sb = wpool.tile([1, B], F32)
            nc.scalar.copy(invT_sb, ivp)

            sps = ppool.tile([B, B], F32, tag="acc", bufs=1, name="sps")
            nc.tensor.matmul(sps, lhsT=gsT_sb[:, 0, :], rhs=gsT_sb[:, 0, :],
                             start=True, stop=False)
            nc.tensor.matmul(sps, lhsT=gsT_sb[:, 1, :], rhs=gsT_sb[:, 1, :],
                             start=False, stop=False)
            nc.tensor.matmul(sps, lhsT=invT_sb, rhs=invT_sb,
                             start=False, stop=True)

            # ---- mask: threshold | topk | topk.T --------------------------
            s_sb = wpool.tile([B, B], F32)
            nc.vector.tensor_copy(s_sb, sps)
            kk = min(k, B)
            if kk > 0:
                cur = s_sb
                rounds = (kk - 1) // 8
                for r in range(rounds):
                    mx = wpool.tile([B, 8], F32, tag="mx", bufs=2)
                    nc.vector.max(out=mx, in_=cur)
                    nxt = wpool.tile([B, B], F32, tag="scur", bufs=2)
                    nc.vector.match_replace(
                        out=nxt, in_to_replace=mx, in_values=cur,
                        imm_value=-10.0,
                    )
                    cur = nxt
                mxf = wpool.tile([B, 8], F32)
                nc.vector.max(out=mxf, in_=cur)
                pos = (kk - 1) % 8
                tmask = wpool.tile([B, B], F32)
                nc.vector.tensor_scalar(
                    tmask, s_sb, mxf[:, pos:pos + 1], None,
                    op0=mybir.AluOpType.is_ge,
                )
                ttp = ppool.tile([B, B], F32, tag="tp", bufs=2)
                nc.tensor.transpose(ttp, tmask, ident)
                # m1 = max(sim > 0.7, topk)   (one fused DVE op)
                m1 = wpool.tile([B, B], F32)
                nc.vector.scalar_tensor_tensor(
                    out=m1, in0=s_sb, scalar=THRESHOLD, in1=tmask,
                    op0=mybir.AluOpType.is_gt, op1=mybir.AluOpType.max,
                )
                m2 = wpool.tile([B, B], F32)
                nc.vector.tensor_tensor(
                    out=m2, in0=m1, in1=ttp, op=mybir.AluOpType.max
                )
                final_mask = m2
            else:
                final_mask = wpool.tile([B, B], F32)
                nc.vector.tensor_scalar(
                    final_mask, s_sb, THRESHOLD, None, op0=mybir.AluOpType.is_gt
                )
            nc.sync.dma_start(mask_d, final_mask)

            # g region: feats[b, 16384:] = g_b/norm_b for this core's rows
            gshp = ppool.tile([RPC, TD], F32, tag="small", bufs=1, name="gshp")
            nc.tensor.matmul(gshp, lhsT=sel_sb, rhs=gs, start=True, stop=True)
            gsh_sb = wpool.tile([RPC, TD], F32)
            nc.scalar.copy(gsh_sb, gshp)
            nc.sync.dma_start(feats_d[:, KC:], gsh_sb)

    nc.compile()
    return nc


def _prep_inputs(global_token, W1_global, b1_global, W2_global, b2_global):
    gt = np.ascontiguousarray(np.asarray(global_token, np.float32))
    W1 = np.asarray(W1_global, np.float32)
    b1 = np.asarray(b1_global, np.float32)
    W2 = np.asarray(W2_global, np.float32)
    b2 = np.asarray(b2_global, np.float32)

    gth = np.ascontiguousarray(gt.T.reshape(KD, 128, B).transpose(1, 0, 2))
    w1h = np.ascontiguousarray(W1.reshape(KD, 128, MLP).transpose(1, 0, 2))
    b1h = np.ascontiguousarray(b1.reshape(KM, 128).T)
    w2h = np.ascontiguousarray(W2.reshape(KM, 128, TD).transpose(1, 0, 2))
    b2r = np.ascontiguousarray(np.broadcast_to(b2[None, :], (B, TD)))
    return gth, w1h, b1h, w2h, b2r


def _make_in_maps(gth, w1h, b1h, w2h, b2r):
    in_maps = []
    for c in range(N_CORES):
        rsel = np.zeros((B, 128), np.float32)
        for p in range(128):
            rsel[c * RPC + p // 16, p] = 1.0
        selm = np.zeros((B, RPC), np.float32)
        for j in range(RPC):
            selm[c * RPC + j, j] = 1.0
        small = np.ascontiguousarray(
            np.concatenate([b2r, rsel, selm], axis=1)
        )
        in_maps.append({
            "gtT": gth, "w1": w1h, "b1": b1h, "w2": w2h, "small": small,
        })
    return in_maps


def kernel(local_features=None, global_token=None,
           W1_local=None, b1_local=None, W2_local=None, b2_local=None,
           W1_global=None, b1_global=None, W2_global=None, b2_global=None,
           k_nearest=10, **_unused):
    k = int(k_nearest)
    if k not in _cache:
        _cache[k] = _build(k)
    nc = _cache[k]

    gth, w1h, b1h, w2h, b2r = _prep_inputs(
        global_token, W1_global, b1_global, W2_global, b2_global
    )

    in_maps = _make_in_maps(gth, w1h, b1h, w2h, b2r)
    res = run_bass_kernel_spmd(nc, in_maps, core_ids=list(range(N_CORES)))
    feats = np.concatenate(
        [res.results[c]["feats"] for c in range(N_CORES)], axis=0
    )
    mask = res.results[0]["mask"]
    return feats, mask
